# revision 1
# baseline (speedup 1.0000x reference)
"""Causal self-attention with RoPE on 8 trn2 NeuronCores (~441us HW).

Sharding: tensor-parallel over heads (Megatron style). 16 heads, 8 cores
-> 2 heads per core. Each core computes q/k/v for its 2 heads, causal
attention, and a partial output projection against its w_o column slice.
Host sums the 8 partial outputs (the Megatron all-reduce, done at gather).

Device-side design (bf16 compute, fp32 PSUM accumulation throughout):
 - xT [C, B*T] bf16: x pre-transposed on host so the QKV contraction dim
   (c) is on SBUF partitions; no on-device transpose of x.
 - w_qk packed per head into even/odd-dim column blocks [QE|QO|KE|KO];
   QKV matmuls produce q/k directly in [d, t] layout, head-stacked so
   RoPE runs full-128-partition DVE tensor_tensor ops (6 per tile).
   PSUM is freed via one wide ScalarE copy per tile; rope reads SBUF.
 - v in natural [t, d] layout (same x tiles, shared j/c loop), stored
   with a ones column per head: AV then yields y[tq, 0:128] AND the
   softmax denominator y[:, 128] from a single accumulated matmul.
 - Post-rope q/k repacked (SBUF->SBUF DMA) into per-head [d, t] tensors
   so scores are single K=128 matmuls: scoresT[ts, tq] = khat.T @ qhat.
 - Softmax: no max-subtraction (scores ~N(0,1)); exp on ScalarE with the
   1/sqrt(D) scale folded in, emitted over paired 1024-wide tq blocks to
   amortize per-instruction overhead; causal masking only on
   diagonal-touching tiles via 0/1 bf16 mask multiply.
 - Normalize with reciprocal + per-partition tensor_scalar, PE-transpose
   y -> yT, output projection accumulates both heads in PSUM, ScalarE/DVE
   bounce to SBUF, DMA out as a partial [B*T, C] f32 result.

Compile recipe (this container): bacc.Bacc("TRN2") + TileContext +
nc.finalize() before run_bass_kernel_spmd (bacc legalizes multi-wait
instructions; raw bass.Bass fails walrus codegen here).
"""

import math

import numpy as np

B, T, C, H = 2, 2048, 2048, 16
D = C // H  # 128
NCORES = 8
HPC = H // NCORES  # heads per core = 2
N = B * T  # 4096 token rows
TT = T // 128  # 16 t-tiles per batch
NB = T // 512  # 4 n/tq blocks of 512 per batch
CT = C // 128  # 16 contraction tiles

_COMPILED = None


def _build():
    import concourse.bacc as bacc
    import concourse.mybir as mybir
    import concourse.tile as tile
    from concourse.masks import make_identity

    f32 = mybir.dt.float32
    bf16 = mybir.dt.bfloat16

    nc = bacc.Bacc("TRN2", target_bir_lowering=False, debug=False)
    xT = nc.declare_dram_parameter("xT", [C, N], bf16, isOutput=False)
    w_qk = nc.declare_dram_parameter("w_qk", [C, 4 * D], bf16, isOutput=False)
    w_v = nc.declare_dram_parameter("w_v", [C, HPC * D], bf16, isOutput=False)
    w_o = nc.declare_dram_parameter("w_o", [HPC * D, C], bf16, isOutput=False)
    cos2 = nc.declare_dram_parameter("cos2", [D, N], bf16, isOutput=False)
    sin2 = nc.declare_dram_parameter("sin2", [D, N], bf16, isOutput=False)
    # masks: 4 variants [128,1024] (diag in left half, right half ones)
    # then 4 variants [128,512] (diag within the single block)
    masks = nc.declare_dram_parameter("masks", [128, 4 * 1024 + 4 * 512], bf16, isOutput=False)
    out_p = nc.declare_dram_parameter("out_p", [N, C], f32, isOutput=True)

    SCALE = 1.0 / math.sqrt(D)
    VW = HPC * D + 2 * HPC  # 260: per t-tile v storage [v_h0|1|pad|v_h1|1|pad]

    with tile.TileContext(nc) as tc:
        with (
            tc.tile_pool(name="wpool", bufs=1) as wpool,
            tc.tile_pool(name="xpool", bufs=6) as xpool,
            tc.tile_pool(name="eo", bufs=4) as eopool,
            tc.tile_pool(name="ropetmp", bufs=6) as tmppool,
            tc.tile_pool(name="vsb", bufs=1) as vpool,
            tc.tile_pool(name="expp", bufs=18) as exppool,
            tc.tile_pool(name="ysb", bufs=4) as ypool,
            tc.tile_pool(name="rsb", bufs=8) as rpool,
            tc.tile_pool(name="yts", bufs=3) as ytpool,
            tc.tile_pool(name="pbig", bufs=2, space="PSUM") as pbig,
            tc.tile_pool(name="paux", bufs=4, space="PSUM") as paux,
        ):
            # ---- resident weights / constants ----
            wqk_sb = wpool.tile([128, CT * 512], bf16, tag="wqk")
            nc.sync.dma_start(
                out=wqk_sb[:, :].rearrange("p (kt e) -> p kt e", kt=CT),
                in_=w_qk.rearrange("(kt p) e -> p kt e", p=128),
            )
            wv_sb = wpool.tile([128, CT * 256], bf16, tag="wv")
            nc.sync.dma_start(
                out=wv_sb[:, :].rearrange("p (kt e) -> p kt e", kt=CT),
                in_=w_v.rearrange("(kt p) e -> p kt e", p=128),
            )
            wo_sb = wpool.tile([128, HPC * C], bf16, tag="wo")
            nc.sync.dma_start(
                out=wo_sb[:, :].rearrange("p (kt o) -> p kt o", kt=HPC),
                in_=w_o.rearrange("(kt p) o -> p kt o", p=128),
            )
            cos_sb = wpool.tile([128, N], bf16, tag="cos")
            nc.sync.dma_start(out=cos_sb[:, :], in_=cos2[:, :])
            sin_sb = wpool.tile([128, N], bf16, tag="sin")
            nc.sync.dma_start(out=sin_sb[:, :], in_=sin2[:, :])
            mask_sb = wpool.tile([128, 4 * 1024 + 4 * 512], bf16, tag="mask")
            nc.sync.dma_start(out=mask_sb[:, :], in_=masks[:, :])
            ident = wpool.tile([128, 128], bf16, tag="ident")
            make_identity(nc, ident[:, :])

            v_sb = vpool.tile([128, TT * VW], bf16, tag="vsb")
            for tt in range(TT):
                for h in range(HPC):
                    col = tt * VW + h * 130 + 128
                    nc.vector.memset(v_sb[:, col : col + 1], 1.0)

            for b in range(B):
                n0 = b * T

                # ---- phase QK: q,k projection in [d, t] layout + RoPE ----
                # Two 2-bank psum tiles per j: [QE | QO] and [KE | KO].
                qe2 = eopool.tile([128, T], bf16, tag="eo", name="qe2")
                qo2 = eopool.tile([128, T], bf16, tag="eo", name="qo2")
                ke2 = eopool.tile([128, T], bf16, tag="eo", name="ke2")
                ko2 = eopool.tile([128, T], bf16, tag="eo", name="ko2")
                rot = [(qe2, qo2), (ke2, ko2)]
                qhat = [eopool.tile([128, T], bf16, tag="qh", name=f"qhat{_h}") for _h in range(HPC)]
                khat = [eopool.tile([128, T], bf16, tag="qh", name=f"khat{_h}") for _h in range(HPC)]
                for j in range(NB):
                    js = slice(j * 512, (j + 1) * 512)
                    ps_q = pbig.tile([128, 1024], f32, tag="big", name="ps_q")
                    ps_k = pbig.tile([128, 1024], f32, tag="big", name="ps_k")
                    ps_v = [paux.tile([128, 256], f32, tag="aux", name=f"ps_v{_p}") for _p in range(4)]
                    for c in range(CT):
                        xt = xpool.tile([128, 512], bf16, tag="xt")
                        nc.gpsimd.dma_start(
                            out=xt[:, :],
                            in_=xT[c * 128 : (c + 1) * 128, n0 + j * 512 : n0 + (j + 1) * 512],
                        )
                        for part in range(4):  # QE, QO, KE, KO
                            dst = (ps_q, ps_q, ps_k, ps_k)[part]
                            off = (0, 512, 0, 512)[part]
                            wsl = wqk_sb[:, c * 512 + part * 128 : c * 512 + (part + 1) * 128]
                            nc.tensor.matmul(
                                dst[:, off : off + 512],
                                wsl,
                                xt[:, :],
                                start=(c == 0),
                                stop=(c == CT - 1),
                            )
                        for tl in range(4):
                            nc.tensor.matmul(
                                ps_v[tl][:, :],
                                xt[:, tl * 128 : (tl + 1) * 128],
                                wv_sb[:, c * 256 : (c + 1) * 256],
                                start=(c == 0),
                                stop=(c == CT - 1),
                            )
                    ce = cos_sb[:, n0 + j * 512 : n0 + (j + 1) * 512]
                    se = sin_sb[:, n0 + j * 512 : n0 + (j + 1) * 512]
                    # One wide ACT copy per psum tile frees the banks fast;
                    # rope then runs from SBUF off the PE critical path.
                    for qk in range(2):  # 0 = q, 1 = k
                        pc = tmppool.tile([128, 1024], f32, tag="rt", name=f"pc{qk}")
                        nc.scalar.copy(pc[:, :], (ps_q, ps_k)[qk][:, :])
                        E_sb, O_sb = pc[:, 0:512], pc[:, 512:1024]
                        dst_e, dst_o = rot[qk]
                        t1 = tmppool.tile([128, 512], f32, tag="rt2")
                        t2 = tmppool.tile([128, 512], f32, tag="rt2")
                        nc.vector.tensor_mul(t1[:, :], E_sb, ce)
                        nc.vector.tensor_mul(t2[:, :], O_sb, se)
                        nc.vector.tensor_sub(dst_e[:, js], t1[:, :], t2[:, :])
                        t3 = tmppool.tile([128, 512], f32, tag="rt2")
                        t4 = tmppool.tile([128, 512], f32, tag="rt2")
                        nc.vector.tensor_mul(t3[:, :], E_sb, se)
                        nc.vector.tensor_mul(t4[:, :], O_sb, ce)
                        nc.vector.tensor_add(dst_o[:, js], t3[:, :], t4[:, :])
                    for tl in range(4):
                        tt = j * 4 + tl
                        base = tt * VW
                        for h in range(HPC):
                            nc.vector.tensor_copy(
                                v_sb[:, base + h * 130 : base + h * 130 + 128],
                                ps_v[tl][:, h * 128 : (h + 1) * 128],
                            )
                    for h in range(HPC):
                        hb = 64 * h
                        nc.sync.dma_start(out=qhat[h][0:64, js], in_=qe2[hb : hb + 64, js])
                        nc.sync.dma_start(out=qhat[h][64:128, js], in_=qo2[hb : hb + 64, js])
                        nc.sync.dma_start(out=khat[h][0:64, js], in_=ke2[hb : hb + 64, js])
                        nc.sync.dma_start(out=khat[h][64:128, js], in_=ko2[hb : hb + 64, js])

                # ---- attention per head: paired tq blocks (jlo, jhi) share
                # one [128,1024] score psum + one wide exp instruction ----
                yT = [eopool.tile([128, T], bf16, tag="yt", name=f"yT{_h}") for _h in range(HPC)]
                for h in range(HPC):
                    for jp in range(NB // 2):
                        jlo, jhi = 2 * jp, 2 * jp + 1
                        exp_of = {}  # i -> (tile, base col of jlo half or None)
                        for i in range(4 * jhi + 4):
                            isl = slice(i * 128, (i + 1) * 128)
                            combined = i <= 4 * jlo + 3
                            sc = pbig.tile([128, 1024], f32, tag="big", name="sc")
                            ex = exppool.tile([128, 1024], bf16, tag="ex")
                            if combined:
                                nc.tensor.matmul(
                                    sc[:, 0:512], khat[h][:, isl],
                                    qhat[h][:, jlo * 512 : (jlo + 1) * 512],
                                    start=True, stop=True,
                                )
                                nc.tensor.matmul(
                                    sc[:, 512:1024], khat[h][:, isl],
                                    qhat[h][:, jhi * 512 : (jhi + 1) * 512],
                                    start=True, stop=True,
                                )
                                nc.scalar.activation(
                                    ex[:, :], sc[:, :],
                                    mybir.ActivationFunctionType.Exp, scale=SCALE,
                                )
                                p = i - 4 * jlo
                                if p >= 0:
                                    nc.vector.tensor_mul(
                                        ex[:, :], ex[:, :],
                                        mask_sb[:, p * 1024 : (p + 1) * 1024],
                                    )
                                exp_of[i] = (ex, 0)
                            else:
                                nc.tensor.matmul(
                                    sc[:, 0:512], khat[h][:, isl],
                                    qhat[h][:, jhi * 512 : (jhi + 1) * 512],
                                    start=True, stop=True,
                                )
                                nc.scalar.activation(
                                    ex[:, 0:512], sc[:, 0:512],
                                    mybir.ActivationFunctionType.Exp, scale=SCALE,
                                )
                                p = i - 4 * jhi
                                if p >= 0:
                                    nc.vector.tensor_mul(
                                        ex[:, 0:512], ex[:, 0:512],
                                        mask_sb[:, 4096 + p * 512 : 4096 + (p + 1) * 512],
                                    )
                                exp_of[i] = (ex, None)

                        for j in (jlo, jhi):
                            half = 0 if j == jlo else 512
                            y_ps = [paux.tile([128, 129], f32, tag="aux", name=f"y_ps{_p}") for _p in range(4)]
                            for tau in range(4):
                                g = 4 * j + tau
                                for i in range(g + 1):
                                    ex, base = exp_of[i]
                                    col = (half if base == 0 else 0) + tau * 128
                                    nc.tensor.matmul(
                                        y_ps[tau][:, :],
                                        ex[:, col : col + 128],
                                        v_sb[:, i * VW + h * 130 : i * VW + h * 130 + 129],
                                        start=(i == 0),
                                        stop=(i == g),
                                    )
                            for tau in range(4):
                                g = 4 * j + tau
                                r = rpool.tile([128, 1], f32, tag="r")
                                nc.vector.reciprocal(r[:, :], y_ps[tau][:, 128:129])
                                y_sb = ypool.tile([128, 128], bf16, tag="y")
                                nc.vector.tensor_scalar_mul(
                                    y_sb[:, :], y_ps[tau][:, 0:128], r[:, 0:1]
                                )
                                yt_ps = paux.tile([128, 128], bf16, tag="aux")
                                nc.tensor.transpose(yt_ps[:, :], y_sb[:, :], ident[:, :])
                                nc.vector.tensor_copy(
                                    yT[h][:, g * 128 : (g + 1) * 128], yt_ps[:, :]
                                )

                # ---- output projection (partial over this core's heads) ----
                for tt in range(TT):
                    tsl = slice(tt * 128, (tt + 1) * 128)
                    for obp in range(2):  # pairs of 512-wide o blocks
                        o_ps = pbig.tile([128, 1024], f32, tag="big", name="o_ps")
                        for ob in (2 * obp, 2 * obp + 1):
                            off = (ob - 2 * obp) * 512
                            for h in range(HPC):
                                nc.tensor.matmul(
                                    o_ps[:, off : off + 512],
                                    yT[h][:, tsl],
                                    wo_sb[:, h * C + ob * 512 : h * C + (ob + 1) * 512],
                                    start=(h == 0),
                                    stop=(h == HPC - 1),
                                )
                        yo = ytpool.tile([128, 1024], f32, tag="yo")
                        nc.vector.tensor_copy(yo[:, :], o_ps[:, :])
                        nc.sync.dma_start(
                            out=out_p[n0 + tt * 128 : n0 + (tt + 1) * 128, obp * 1024 : (obp + 1) * 1024],
                            in_=yo[:, :],
                        )
    nc.finalize()
    return nc


def _prep_inputs(x, w_qkv, w_o, rope_cos, rope_sin):
    import ml_dtypes

    bf = ml_dtypes.bfloat16
    xTh = np.ascontiguousarray(x.reshape(N, C).T).astype(bf)
    cosT = np.ascontiguousarray(rope_cos.T)  # [64, T]
    sinT = np.ascontiguousarray(rope_sin.T)
    cos2 = np.tile(np.concatenate([cosT, cosT], 0), (1, B)).astype(bf)
    sin2 = np.tile(np.concatenate([sinT, sinT], 0), (1, B)).astype(bf)

    r = np.arange(128)[:, None]
    c = np.arange(512)[None, :]
    singles = [((c - r) >= 128 * p).astype(np.float32) for p in range(4)]
    ones512 = np.ones((128, 512), dtype=np.float32)
    combos = [np.concatenate([s, ones512], 1) for s in singles]
    mk = np.concatenate(combos + singles, axis=1).astype(bf)

    ev = np.arange(0, D, 2)
    od = np.arange(1, D, 2)
    in_maps = []
    for m in range(NCORES):
        h0, h1 = 2 * m, 2 * m + 1
        # blocks QE|QO|KE|KO; within each, cols = [head0 dims | head1 dims]
        QE = np.concatenate([w_qkv[h0 * D + ev, :], w_qkv[h1 * D + ev, :]], 0).T
        QO = np.concatenate([w_qkv[h0 * D + od, :], w_qkv[h1 * D + od, :]], 0).T
        KE = np.concatenate([w_qkv[C + h0 * D + ev, :], w_qkv[C + h1 * D + ev, :]], 0).T
        KO = np.concatenate([w_qkv[C + h0 * D + od, :], w_qkv[C + h1 * D + od, :]], 0).T
        wqk_m = np.ascontiguousarray(np.concatenate([QE, QO, KE, KO], 1)).astype(bf)
        wv_m = np.ascontiguousarray(
            w_qkv[2 * C + 2 * m * D : 2 * C + (2 * m + 2) * D, :].T
        ).astype(bf)
        wo_m = np.ascontiguousarray(w_o[:, 2 * m * D : (2 * m + 2) * D].T).astype(bf)
        in_maps.append(
            {
                "xT": xTh,
                "w_qk": wqk_m,
                "w_v": wv_m,
                "w_o": wo_m,
                "cos2": cos2,
                "sin2": sin2,
                "masks": np.ascontiguousarray(mk),
            }
        )
    return in_maps


def kernel(x, w_qkv, w_o, rope_cos, rope_sin, _trace=False):
    global _COMPILED
    x = np.asarray(x, dtype=np.float32)
    w_qkv = np.asarray(w_qkv, dtype=np.float32)
    w_o = np.asarray(w_o, dtype=np.float32)
    rope_cos = np.asarray(rope_cos, dtype=np.float32)
    rope_sin = np.asarray(rope_sin, dtype=np.float32)

    from concourse.bass_utils import run_bass_kernel_spmd

    if _COMPILED is None:
        _COMPILED = _build()
    nc = _COMPILED
    in_maps = _prep_inputs(x, w_qkv, w_o, rope_cos, rope_sin)
    res = run_bass_kernel_spmd(
        nc, in_maps, core_ids=list(range(NCORES)), trace=_trace
    )
    out = np.zeros((N, C), dtype=np.float32)
    for m in range(NCORES):
        out += res.results[m]["out_p"]
    kernel._last_results = res
    return out.reshape(B, T, C)



# revision 6
# speedup vs baseline: 1.1288x; 1.1288x over previous
"""Causal self-attention with RoPE on 8 trn2 NeuronCores.

Sharding: tensor-parallel over heads (Megatron style). 16 heads, 8 cores
-> 2 heads per core. Each core computes q/k/v for its 2 heads, causal
attention, and a partial output projection against its w_o column slice.
Host sums the 8 partial outputs (the Megatron all-reduce, done at gather).

v3 = baseline structure + low-risk DMA/dtype changes:
 - x tiles arrive via 2 batched 3D DMAs per j-block (8 c-tiles each)
   instead of 16 small DMAs; initial weight DMAs split into chunks and
   ordered just-in-time (wqk chunks -> cos/sin -> wv -> masks -> wo) so
   the first matmul starts early.
 - partial outputs written bf16 (host accumulates in f32).
 - masks shrunk to the 4 [128,512] diagonal variants (the all-ones half
   of combined tiles is never multiplied).
 - cos/sin not duplicated per batch; rope runs in bf16.
 - softmax normalize moved to ScalarE (activation Copy with
   per-partition scale), freeing DVE.
"""

import math

import numpy as np

B, T, C, H = 2, 2048, 2048, 16
D = C // H  # 128
NCORES = 8
HPC = H // NCORES  # heads per core = 2
N = B * T  # 4096 token rows
TT = T // 128  # 16 t-tiles per batch
NB = T // 512  # 4 n/tq blocks of 512 per batch
CT = C // 128  # 16 contraction tiles

_COMPILED = None


def _build():
    import concourse.bacc as bacc
    import concourse.mybir as mybir
    import concourse.tile as tile
    from concourse.masks import make_identity

    f32 = mybir.dt.float32
    bf16 = mybir.dt.bfloat16

    nc = bacc.Bacc("TRN2", target_bir_lowering=False, debug=False)
    xT = nc.declare_dram_parameter("xT", [C, N], bf16, isOutput=False)
    w_qk = nc.declare_dram_parameter("w_qk", [C, 4 * D], bf16, isOutput=False)
    w_v = nc.declare_dram_parameter("w_v", [C, HPC * D], bf16, isOutput=False)
    w_o = nc.declare_dram_parameter("w_o", [HPC * D, C], bf16, isOutput=False)
    cos2 = nc.declare_dram_parameter("cos2", [D, T], bf16, isOutput=False)
    sin2 = nc.declare_dram_parameter("sin2", [D, T], bf16, isOutput=False)
    # masks: 4 diagonal variants [128,512]; variant p keeps (c - r) >= 128*p
    masks = nc.declare_dram_parameter("masks", [128, 4 * 512], bf16, isOutput=False)
    out_p = nc.declare_dram_parameter("out_p", [N, C], bf16, isOutput=True)

    SCALE = 1.0 / math.sqrt(D)
    VW = HPC * D + 2 * HPC  # 260: per t-tile v storage [v_h0|1|pad|v_h1|1|pad]

    with tile.TileContext(nc) as tc:
        with (
            tc.tile_pool(name="wpool", bufs=1) as wpool,
            tc.tile_pool(name="xpool", bufs=2) as xpool,
            tc.tile_pool(name="eo", bufs=4) as eopool,
            tc.tile_pool(name="ropetmp", bufs=6) as tmppool,
            tc.tile_pool(name="vsb", bufs=1) as vpool,
            tc.tile_pool(name="expp", bufs=18) as exppool,
            tc.tile_pool(name="ysb", bufs=4) as ypool,
            tc.tile_pool(name="rsb", bufs=8) as rpool,
            tc.tile_pool(name="yts", bufs=3) as ytpool,
            tc.tile_pool(name="pbig", bufs=2, space="PSUM") as pbig,
            tc.tile_pool(name="paux", bufs=4, space="PSUM") as paux,
        ):
            # ---- resident weights / constants, chunked + JIT ordered ----
            wqk_sb = wpool.tile([128, CT * 512], bf16, tag="wqk")
            for ch in range(4):  # 4 c-tiles per chunk
                c0 = ch * 4
                nc.sync.dma_start(
                    out=wqk_sb[:, c0 * 512 : (c0 + 4) * 512].rearrange(
                        "p (kt e) -> p kt e", kt=4
                    ),
                    in_=w_qk[c0 * 128 : (c0 + 4) * 128, :].rearrange(
                        "(kt p) e -> p kt e", p=128
                    ),
                )
            cos_sb = wpool.tile([128, T], bf16, tag="cos")
            nc.sync.dma_start(out=cos_sb[:, :], in_=cos2[:, :])
            sin_sb = wpool.tile([128, T], bf16, tag="sin")
            nc.sync.dma_start(out=sin_sb[:, :], in_=sin2[:, :])
            wv_sb = wpool.tile([128, CT * 256], bf16, tag="wv")
            for ch in range(2):
                c0 = ch * 8
                nc.sync.dma_start(
                    out=wv_sb[:, c0 * 256 : (c0 + 8) * 256].rearrange(
                        "p (kt e) -> p kt e", kt=8
                    ),
                    in_=w_v[c0 * 128 : (c0 + 8) * 128, :].rearrange(
                        "(kt p) e -> p kt e", p=128
                    ),
                )
            mask_sb = wpool.tile([128, 4 * 512], bf16, tag="mask")
            nc.sync.dma_start(out=mask_sb[:, :], in_=masks[:, :])
            wo_sb = wpool.tile([128, HPC * C], bf16, tag="wo")
            nc.sync.dma_start(
                out=wo_sb[:, :].rearrange("p (kt o) -> p kt o", kt=HPC),
                in_=w_o.rearrange("(kt p) o -> p kt o", p=128),
            )
            ident = wpool.tile([128, 128], bf16, tag="ident")
            make_identity(nc, ident[:, :])

            v_sb = vpool.tile([128, TT * VW], bf16, tag="vsb")
            for tt in range(TT):
                for h in range(HPC):
                    col = tt * VW + h * 130 + 128
                    nc.vector.memset(v_sb[:, col : col + 1], 1.0)

            for b in range(B):
                n0 = b * T

                # ---- phase QK: q,k projection in [d, t] layout + RoPE ----
                qe2 = eopool.tile([128, T], bf16, tag="eo", name="qe2")
                qo2 = eopool.tile([128, T], bf16, tag="eo", name="qo2")
                ke2 = eopool.tile([128, T], bf16, tag="eo", name="ke2")
                ko2 = eopool.tile([128, T], bf16, tag="eo", name="ko2")
                rot = [(qe2, qo2), (ke2, ko2)]
                qhat = [eopool.tile([128, T], bf16, tag="qh", name=f"qhat{_h}") for _h in range(HPC)]
                khat = [eopool.tile([128, T], bf16, tag="qh", name=f"khat{_h}") for _h in range(HPC)]
                for j in range(NB):
                    js = slice(j * 512, (j + 1) * 512)
                    xt = xpool.tile([128, CT * 512], bf16, tag="xt")
                    for ch in range(2):
                        c0 = ch * 8
                        nc.gpsimd.dma_start(
                            out=xt[:, c0 * 512 : (c0 + 8) * 512].rearrange(
                                "p (c n) -> p c n", c=8
                            ),
                            in_=xT[
                                c0 * 128 : (c0 + 8) * 128,
                                n0 + j * 512 : n0 + (j + 1) * 512,
                            ].rearrange("(c p) n -> p c n", p=128),
                        )
                    ps_q = pbig.tile([128, 1024], f32, tag="big", name="ps_q")
                    ps_k = pbig.tile([128, 1024], f32, tag="big", name="ps_k")
                    ps_v = [paux.tile([128, 256], f32, tag="aux", name=f"ps_v{_p}") for _p in range(4)]
                    for c in range(CT):
                        xc = xt[:, c * 512 : (c + 1) * 512]
                        for part in range(4):  # QE, QO, KE, KO
                            dst = (ps_q, ps_q, ps_k, ps_k)[part]
                            off = (0, 512, 0, 512)[part]
                            wsl = wqk_sb[:, c * 512 + part * 128 : c * 512 + (part + 1) * 128]
                            nc.tensor.matmul(
                                dst[:, off : off + 512],
                                wsl,
                                xc,
                                start=(c == 0),
                                stop=(c == CT - 1),
                            )
                        for tl in range(4):
                            nc.tensor.matmul(
                                ps_v[tl][:, :],
                                xt[:, c * 512 + tl * 128 : c * 512 + (tl + 1) * 128],
                                wv_sb[:, c * 256 : (c + 1) * 256],
                                start=(c == 0),
                                stop=(c == CT - 1),
                            )
                    ce = cos_sb[:, js]
                    se = sin_sb[:, js]
                    # One wide ACT copy per psum tile frees the banks fast;
                    # rope then runs from SBUF off the PE critical path.
                    for qk in range(2):  # 0 = q, 1 = k
                        pc = tmppool.tile([128, 1024], bf16, tag="rt", name=f"pc{qk}")
                        nc.scalar.copy(pc[:, :], (ps_q, ps_k)[qk][:, :])
                        E_sb, O_sb = pc[:, 0:512], pc[:, 512:1024]
                        dst_e, dst_o = rot[qk]
                        t1 = tmppool.tile([128, 512], bf16, tag="rt2")
                        t2 = tmppool.tile([128, 512], bf16, tag="rt2")
                        nc.vector.tensor_mul(t1[:, :], E_sb, ce)
                        nc.vector.tensor_mul(t2[:, :], O_sb, se)
                        nc.vector.tensor_sub(dst_e[:, js], t1[:, :], t2[:, :])
                        t3 = tmppool.tile([128, 512], bf16, tag="rt2")
                        t4 = tmppool.tile([128, 512], bf16, tag="rt2")
                        nc.vector.tensor_mul(t3[:, :], E_sb, se)
                        nc.vector.tensor_mul(t4[:, :], O_sb, ce)
                        nc.vector.tensor_add(dst_o[:, js], t3[:, :], t4[:, :])
                    for tl in range(4):
                        tt = j * 4 + tl
                        base = tt * VW
                        for h in range(HPC):
                            nc.vector.tensor_copy(
                                v_sb[:, base + h * 130 : base + h * 130 + 128],
                                ps_v[tl][:, h * 128 : (h + 1) * 128],
                            )
                    for h in range(HPC):
                        hb = 64 * h
                        nc.sync.dma_start(out=qhat[h][0:64, js], in_=qe2[hb : hb + 64, js])
                        nc.sync.dma_start(out=qhat[h][64:128, js], in_=qo2[hb : hb + 64, js])
                        nc.sync.dma_start(out=khat[h][0:64, js], in_=ke2[hb : hb + 64, js])
                        nc.sync.dma_start(out=khat[h][64:128, js], in_=ko2[hb : hb + 64, js])

                # ---- attention per head: paired tq blocks (jlo, jhi) share
                # one [128,1024] score psum + one wide exp instruction ----
                yT = [eopool.tile([128, T], bf16, tag="yt", name=f"yT{_h}") for _h in range(HPC)]
                for h in range(HPC):
                    for jp in range(NB // 2):
                        jlo, jhi = 2 * jp, 2 * jp + 1
                        exp_of = {}  # i -> (tile, combined flag)
                        for i in range(4 * jhi + 4):
                            isl = slice(i * 128, (i + 1) * 128)
                            combined = i <= 4 * jlo + 3
                            sc = pbig.tile([128, 1024], f32, tag="big", name="sc")
                            ex = exppool.tile([128, 1024], bf16, tag="ex")
                            if combined:
                                nc.tensor.matmul(
                                    sc[:, 0:512], khat[h][:, isl],
                                    qhat[h][:, jlo * 512 : (jlo + 1) * 512],
                                    start=True, stop=True,
                                )
                                nc.tensor.matmul(
                                    sc[:, 512:1024], khat[h][:, isl],
                                    qhat[h][:, jhi * 512 : (jhi + 1) * 512],
                                    start=True, stop=True,
                                )
                                nc.scalar.activation(
                                    ex[:, :], sc[:, :],
                                    mybir.ActivationFunctionType.Exp, scale=SCALE,
                                )
                                p = i - 4 * jlo
                                if p >= 0:
                                    nc.vector.tensor_mul(
                                        ex[:, 0:512], ex[:, 0:512],
                                        mask_sb[:, p * 512 : (p + 1) * 512],
                                    )
                                exp_of[i] = (ex, True)
                            else:
                                nc.tensor.matmul(
                                    sc[:, 0:512], khat[h][:, isl],
                                    qhat[h][:, jhi * 512 : (jhi + 1) * 512],
                                    start=True, stop=True,
                                )
                                nc.scalar.activation(
                                    ex[:, 0:512], sc[:, 0:512],
                                    mybir.ActivationFunctionType.Exp, scale=SCALE,
                                )
                                p = i - 4 * jhi
                                if p >= 0:
                                    nc.vector.tensor_mul(
                                        ex[:, 0:512], ex[:, 0:512],
                                        mask_sb[:, p * 512 : (p + 1) * 512],
                                    )
                                exp_of[i] = (ex, False)

                        for jj in (jlo, jhi):
                            half = 0 if jj == jlo else 512
                            y_ps = [paux.tile([128, 129], f32, tag="aux", name=f"y_ps{_p}") for _p in range(4)]
                            for tau in range(4):
                                g = 4 * jj + tau
                                for i in range(g + 1):
                                    ex, comb = exp_of[i]
                                    col = (half if comb else 0) + tau * 128
                                    nc.tensor.matmul(
                                        y_ps[tau][:, :],
                                        ex[:, col : col + 128],
                                        v_sb[:, i * VW + h * 130 : i * VW + h * 130 + 129],
                                        start=(i == 0),
                                        stop=(i == g),
                                    )
                            for tau in range(4):
                                g = 4 * jj + tau
                                r = rpool.tile([128, 1], f32, tag="r")
                                nc.vector.reciprocal(r[:, :], y_ps[tau][:, 128:129])
                                y_sb = ypool.tile([128, 128], bf16, tag="y")
                                nc.scalar.mul(y_sb[:, :], y_ps[tau][:, 0:128], r[:, 0:1])
                                yt_ps = paux.tile([128, 128], bf16, tag="aux")
                                nc.tensor.transpose(yt_ps[:, :], y_sb[:, :], ident[:, :])
                                nc.vector.tensor_copy(
                                    yT[h][:, g * 128 : (g + 1) * 128], yt_ps[:, :]
                                )

                # ---- output projection (partial over this core's heads) ----
                for tt in range(TT):
                    tsl = slice(tt * 128, (tt + 1) * 128)
                    stage = ytpool.tile([128, C], bf16, tag="yo")
                    for obp in range(2):  # pairs of 512-wide o blocks
                        o_ps = pbig.tile([128, 1024], f32, tag="big", name="o_ps")
                        for ob in (2 * obp, 2 * obp + 1):
                            off = (ob - 2 * obp) * 512
                            for h in range(HPC):
                                nc.tensor.matmul(
                                    o_ps[:, off : off + 512],
                                    yT[h][:, tsl],
                                    wo_sb[:, h * C + ob * 512 : h * C + (ob + 1) * 512],
                                    start=(h == 0),
                                    stop=(h == HPC - 1),
                                )
                        nc.vector.tensor_copy(
                            stage[:, obp * 1024 : (obp + 1) * 1024], o_ps[:, :]
                        )
                    nc.gpsimd.dma_start(
                        out=out_p[n0 + tt * 128 : n0 + (tt + 1) * 128, :],
                        in_=stage[:, :],
                    )
    nc.finalize()
    return nc


def _prep_inputs(x, w_qkv, w_o, rope_cos, rope_sin):
    import ml_dtypes

    bf = ml_dtypes.bfloat16
    xTh = np.ascontiguousarray(x.reshape(N, C).T).astype(bf)
    cosT = np.ascontiguousarray(rope_cos.T)  # [64, T]
    sinT = np.ascontiguousarray(rope_sin.T)
    cos2 = np.concatenate([cosT, cosT], 0).astype(bf)  # [128, T]
    sin2 = np.concatenate([sinT, sinT], 0).astype(bf)

    r = np.arange(128)[:, None]
    c = np.arange(512)[None, :]
    singles = [((c - r) >= 128 * p).astype(np.float32) for p in range(4)]
    mk = np.concatenate(singles, axis=1).astype(bf)

    ev = np.arange(0, D, 2)
    od = np.arange(1, D, 2)
    in_maps = []
    for m in range(NCORES):
        h0, h1 = 2 * m, 2 * m + 1
        # blocks QE|QO|KE|KO; within each, cols = [head0 dims | head1 dims]
        QE = np.concatenate([w_qkv[h0 * D + ev, :], w_qkv[h1 * D + ev, :]], 0).T
        QO = np.concatenate([w_qkv[h0 * D + od, :], w_qkv[h1 * D + od, :]], 0).T
        KE = np.concatenate([w_qkv[C + h0 * D + ev, :], w_qkv[C + h1 * D + ev, :]], 0).T
        KO = np.concatenate([w_qkv[C + h0 * D + od, :], w_qkv[C + h1 * D + od, :]], 0).T
        wqk_m = np.ascontiguousarray(np.concatenate([QE, QO, KE, KO], 1)).astype(bf)
        wv_m = np.ascontiguousarray(
            w_qkv[2 * C + 2 * m * D : 2 * C + (2 * m + 2) * D, :].T
        ).astype(bf)
        wo_m = np.ascontiguousarray(w_o[:, 2 * m * D : (2 * m + 2) * D].T).astype(bf)
        in_maps.append(
            {
                "xT": xTh,
                "w_qk": wqk_m,
                "w_v": wv_m,
                "w_o": wo_m,
                "cos2": cos2,
                "sin2": sin2,
                "masks": np.ascontiguousarray(mk),
            }
        )
    return in_maps


def kernel(x, w_qkv, w_o, rope_cos, rope_sin, _trace=False):
    global _COMPILED
    x = np.asarray(x, dtype=np.float32)
    w_qkv = np.asarray(w_qkv, dtype=np.float32)
    w_o = np.asarray(w_o, dtype=np.float32)
    rope_cos = np.asarray(rope_cos, dtype=np.float32)
    rope_sin = np.asarray(rope_sin, dtype=np.float32)

    from concourse.bass_utils import run_bass_kernel_spmd

    if _COMPILED is None:
        _COMPILED = _build()
    nc = _COMPILED
    in_maps = _prep_inputs(x, w_qkv, w_o, rope_cos, rope_sin)
    res = run_bass_kernel_spmd(
        nc, in_maps, core_ids=list(range(NCORES)), trace=_trace
    )
    out = np.zeros((N, C), dtype=np.float32)
    for m in range(NCORES):
        out += res.results[m]["out_p"].astype(np.float32)
    kernel._last_results = res
    return out.reshape(B, T, C)


# revision 7
# speedup vs baseline: 1.1438x; 1.0133x over previous
"""Causal self-attention with RoPE on 8 trn2 NeuronCores.

Sharding: tensor-parallel over heads (Megatron style). 16 heads, 8 cores
-> 2 heads per core. Each core computes q/k/v for its 2 heads, causal
attention, and a partial output projection against its w_o column slice.
Host sums the 8 partial outputs (the Megatron all-reduce, done at gather).

v4 = v3 + software-pipelined emission order:
 - Projection splits q / k / v into separate c-loops per j-block so each
   psum evacuation (ScalarE copy + rope + repack) overlaps the next
   sub-phase's matmuls instead of stalling the j boundary.
 - Per-batch schedule P0 P1 P2 A(jp0) P3 O(0..7) A1h0 A1h1+O(8..15):
   attention for the first two tq blocks is emitted as soon as its
   inputs exist, the first half of the output projection covers P3's
   rope/repack latency, and the second half is interleaved into the last
   attention pass, eliminating the serialized o-projection tail.
 - v_sb double-buffered per batch so batch 1's projection does not stall
   on batch 0's attention reads.
 - DMA order: wqk chunks -> wv -> cos/sin -> masks -> wo on sync queue;
   x tiles batched 3D DMAs on gpsimd (j0 split finer for JIT start).
"""

import math

import numpy as np

B, T, C, H = 2, 2048, 2048, 16
D = C // H  # 128
NCORES = 8
HPC = H // NCORES  # heads per core = 2
N = B * T  # 4096 token rows
TT = T // 128  # 16 t-tiles per batch
NB = T // 512  # 4 n/tq blocks of 512 per batch
CT = C // 128  # 16 contraction tiles

_COMPILED = None


def _build():
    import concourse.bacc as bacc
    import concourse.mybir as mybir
    import concourse.tile as tile
    from concourse.masks import make_identity

    f32 = mybir.dt.float32
    bf16 = mybir.dt.bfloat16

    nc = bacc.Bacc("TRN2", target_bir_lowering=False, debug=False)
    xT = nc.declare_dram_parameter("xT", [C, N], bf16, isOutput=False)
    w_qk = nc.declare_dram_parameter("w_qk", [C, 4 * D], bf16, isOutput=False)
    w_v = nc.declare_dram_parameter("w_v", [C, HPC * D], bf16, isOutput=False)
    w_o = nc.declare_dram_parameter("w_o", [HPC * D, C], bf16, isOutput=False)
    cos2 = nc.declare_dram_parameter("cos2", [D, T], bf16, isOutput=False)
    sin2 = nc.declare_dram_parameter("sin2", [D, T], bf16, isOutput=False)
    # masks: 4 diagonal variants [128,512]; variant p keeps (c - r) >= 128*p
    masks = nc.declare_dram_parameter("masks", [128, 4 * 512], bf16, isOutput=False)
    out_p = nc.declare_dram_parameter("out_p", [N, C], bf16, isOutput=True)

    SCALE = 1.0 / math.sqrt(D)
    VW = HPC * D + 2 * HPC  # 260: per t-tile v storage [v_h0|1|pad|v_h1|1|pad]

    with tile.TileContext(nc) as tc:
        with (
            tc.tile_pool(name="wpool", bufs=1) as wpool,
            tc.tile_pool(name="xpool", bufs=2) as xpool,
            tc.tile_pool(name="eo", bufs=4) as eopool,
            tc.tile_pool(name="ropetmp", bufs=6) as tmppool,
            tc.tile_pool(name="vsb", bufs=2) as vpool,
            tc.tile_pool(name="expp", bufs=18) as exppool,
            tc.tile_pool(name="ysb", bufs=4) as ypool,
            tc.tile_pool(name="rsb", bufs=8) as rpool,
            tc.tile_pool(name="yts", bufs=3) as ytpool,
            tc.tile_pool(name="pbig", bufs=2, space="PSUM") as pbig,
            tc.tile_pool(name="paux", bufs=4, space="PSUM") as paux,
        ):
            # ---- resident weights / constants, chunked + JIT ordered ----
            wqk_sb = wpool.tile([128, CT * 512], bf16, tag="wqk")
            for ch in range(4):  # 4 c-tiles per chunk
                c0 = ch * 4
                nc.sync.dma_start(
                    out=wqk_sb[:, c0 * 512 : (c0 + 4) * 512].rearrange(
                        "p (kt e) -> p kt e", kt=4
                    ),
                    in_=w_qk[c0 * 128 : (c0 + 4) * 128, :].rearrange(
                        "(kt p) e -> p kt e", p=128
                    ),
                )
            wv_sb = wpool.tile([128, CT * 256], bf16, tag="wv")
            for ch in range(2):
                c0 = ch * 8
                nc.sync.dma_start(
                    out=wv_sb[:, c0 * 256 : (c0 + 8) * 256].rearrange(
                        "p (kt e) -> p kt e", kt=8
                    ),
                    in_=w_v[c0 * 128 : (c0 + 8) * 128, :].rearrange(
                        "(kt p) e -> p kt e", p=128
                    ),
                )
            cos_sb = wpool.tile([128, T], bf16, tag="cos")
            nc.sync.dma_start(out=cos_sb[:, :], in_=cos2[:, :])
            sin_sb = wpool.tile([128, T], bf16, tag="sin")
            nc.sync.dma_start(out=sin_sb[:, :], in_=sin2[:, :])
            mask_sb = wpool.tile([128, 4 * 512], bf16, tag="mask")
            nc.sync.dma_start(out=mask_sb[:, :], in_=masks[:, :])
            wo_sb = wpool.tile([128, HPC * C], bf16, tag="wo")
            nc.sync.dma_start(
                out=wo_sb[:, :].rearrange("p (kt o) -> p kt o", kt=HPC),
                in_=w_o.rearrange("(kt p) o -> p kt o", p=128),
            )
            ident = wpool.tile([128, 128], bf16, tag="ident")
            make_identity(nc, ident[:, :])

            for b in range(B):
                n0 = b * T

                v_sb = vpool.tile([128, TT * VW], bf16, tag="vsb")
                for tt in range(TT):
                    for h in range(HPC):
                        col = tt * VW + h * 130 + 128
                        nc.vector.memset(v_sb[:, col : col + 1], 1.0)

                qe2 = eopool.tile([128, T], bf16, tag="eo", name="qe2")
                qo2 = eopool.tile([128, T], bf16, tag="eo", name="qo2")
                ke2 = eopool.tile([128, T], bf16, tag="eo", name="ke2")
                ko2 = eopool.tile([128, T], bf16, tag="eo", name="ko2")
                rot = [(qe2, qo2), (ke2, ko2)]
                qhat = [eopool.tile([128, T], bf16, tag="qh", name=f"qhat{_h}") for _h in range(HPC)]
                khat = [eopool.tile([128, T], bf16, tag="qh", name=f"khat{_h}") for _h in range(HPC)]
                yT = [eopool.tile([128, T], bf16, tag="yt", name=f"yT{_h}") for _h in range(HPC)]

                def proj_block(j):
                    js = slice(j * 512, (j + 1) * 512)
                    xt = xpool.tile([128, CT * 512], bf16, tag="xt")
                    nchunk = 4 if (b == 0 and j == 0) else 2
                    cper = CT // nchunk
                    for ch in range(nchunk):
                        c0 = ch * cper
                        nc.gpsimd.dma_start(
                            out=xt[:, c0 * 512 : (c0 + cper) * 512].rearrange(
                                "p (c n) -> p c n", c=cper
                            ),
                            in_=xT[
                                c0 * 128 : (c0 + cper) * 128,
                                n0 + j * 512 : n0 + (j + 1) * 512,
                            ].rearrange("(c p) n -> p c n", p=128),
                        )
                    ce = cos_sb[:, js]
                    se = sin_sb[:, js]
                    for qk in range(2):  # 0 = q, 1 = k
                        ps = pbig.tile([128, 1024], f32, tag="big", name="ps_qk")
                        for c in range(CT):
                            xc = xt[:, c * 512 : (c + 1) * 512]
                            for part in range(2):
                                wsl = wqk_sb[
                                    :,
                                    c * 512
                                    + (2 * qk + part) * 128 : c * 512
                                    + (2 * qk + part + 1) * 128,
                                ]
                                nc.tensor.matmul(
                                    ps[:, part * 512 : (part + 1) * 512],
                                    wsl,
                                    xc,
                                    start=(c == 0),
                                    stop=(c == CT - 1),
                                )
                        pc = tmppool.tile([128, 1024], bf16, tag="rt", name=f"pc{qk}")
                        nc.scalar.copy(pc[:, :], ps[:, :])
                        E_sb, O_sb = pc[:, 0:512], pc[:, 512:1024]
                        dst_e, dst_o = rot[qk]
                        t1 = tmppool.tile([128, 512], bf16, tag="rt2")
                        t2 = tmppool.tile([128, 512], bf16, tag="rt2")
                        nc.vector.tensor_mul(t1[:, :], E_sb, ce)
                        nc.vector.tensor_mul(t2[:, :], O_sb, se)
                        nc.vector.tensor_sub(dst_e[:, js], t1[:, :], t2[:, :])
                        t3 = tmppool.tile([128, 512], bf16, tag="rt2")
                        t4 = tmppool.tile([128, 512], bf16, tag="rt2")
                        nc.vector.tensor_mul(t3[:, :], E_sb, se)
                        nc.vector.tensor_mul(t4[:, :], O_sb, ce)
                        nc.vector.tensor_add(dst_o[:, js], t3[:, :], t4[:, :])
                        hat = qhat if qk == 0 else khat
                        for h in range(HPC):
                            hb = 64 * h
                            nc.sync.dma_start(
                                out=hat[h][0:64, js], in_=dst_e[hb : hb + 64, js]
                            )
                            nc.sync.dma_start(
                                out=hat[h][64:128, js], in_=dst_o[hb : hb + 64, js]
                            )
                    ps_v = [
                        paux.tile([128, 256], f32, tag="aux", name=f"ps_v{_p}")
                        for _p in range(4)
                    ]
                    for c in range(CT):
                        for tl in range(4):
                            nc.tensor.matmul(
                                ps_v[tl][:, :],
                                xt[:, c * 512 + tl * 128 : c * 512 + (tl + 1) * 128],
                                wv_sb[:, c * 256 : (c + 1) * 256],
                                start=(c == 0),
                                stop=(c == CT - 1),
                            )
                    for tl in range(4):
                        tt = j * 4 + tl
                        base = tt * VW
                        for h in range(HPC):
                            nc.vector.tensor_copy(
                                v_sb[:, base + h * 130 : base + h * 130 + 128],
                                ps_v[tl][:, h * 128 : (h + 1) * 128],
                            )

                def oproj_tile(g):
                    tsl = slice(g * 128, (g + 1) * 128)
                    stage = ytpool.tile([128, C], bf16, tag="yo")
                    for obp in range(2):  # pairs of 512-wide o blocks
                        o_ps = pbig.tile([128, 1024], f32, tag="big", name="o_ps")
                        for ob in (2 * obp, 2 * obp + 1):
                            off = (ob - 2 * obp) * 512
                            for h in range(HPC):
                                nc.tensor.matmul(
                                    o_ps[:, off : off + 512],
                                    yT[h][:, tsl],
                                    wo_sb[:, h * C + ob * 512 : h * C + (ob + 1) * 512],
                                    start=(h == 0),
                                    stop=(h == HPC - 1),
                                )
                        nc.vector.tensor_copy(
                            stage[:, obp * 1024 : (obp + 1) * 1024], o_ps[:, :]
                        )
                    nc.gpsimd.dma_start(
                        out=out_p[n0 + g * 128 : n0 + (g + 1) * 128, :],
                        in_=stage[:, :],
                    )

                def attn(jp, h, interleave_oproj):
                    jlo, jhi = 2 * jp, 2 * jp + 1
                    exp_of = {}  # i -> (tile, combined flag)
                    for i in range(4 * jhi + 4):
                        isl = slice(i * 128, (i + 1) * 128)
                        combined = i <= 4 * jlo + 3
                        sc = pbig.tile([128, 1024], f32, tag="big", name="sc")
                        ex = exppool.tile([128, 1024], bf16, tag="ex")
                        if combined:
                            nc.tensor.matmul(
                                sc[:, 0:512], khat[h][:, isl],
                                qhat[h][:, jlo * 512 : (jlo + 1) * 512],
                                start=True, stop=True,
                            )
                            nc.tensor.matmul(
                                sc[:, 512:1024], khat[h][:, isl],
                                qhat[h][:, jhi * 512 : (jhi + 1) * 512],
                                start=True, stop=True,
                            )
                            nc.scalar.activation(
                                ex[:, :], sc[:, :],
                                mybir.ActivationFunctionType.Exp, scale=SCALE,
                            )
                            p = i - 4 * jlo
                            if p >= 0:
                                nc.vector.tensor_mul(
                                    ex[:, 0:512], ex[:, 0:512],
                                    mask_sb[:, p * 512 : (p + 1) * 512],
                                )
                            exp_of[i] = (ex, True)
                        else:
                            nc.tensor.matmul(
                                sc[:, 0:512], khat[h][:, isl],
                                qhat[h][:, jhi * 512 : (jhi + 1) * 512],
                                start=True, stop=True,
                            )
                            nc.scalar.activation(
                                ex[:, 0:512], sc[:, 0:512],
                                mybir.ActivationFunctionType.Exp, scale=SCALE,
                            )
                            p = i - 4 * jhi
                            if p >= 0:
                                nc.vector.tensor_mul(
                                    ex[:, 0:512], ex[:, 0:512],
                                    mask_sb[:, p * 512 : (p + 1) * 512],
                                )
                            exp_of[i] = (ex, False)

                    for jj in (jlo, jhi):
                        half = 0 if jj == jlo else 512
                        y_ps = [
                            paux.tile([128, 129], f32, tag="aux", name=f"y_ps{_p}")
                            for _p in range(4)
                        ]
                        for tau in range(4):
                            g = 4 * jj + tau
                            for i in range(g + 1):
                                ex, comb = exp_of[i]
                                col = (half if comb else 0) + tau * 128
                                nc.tensor.matmul(
                                    y_ps[tau][:, :],
                                    ex[:, col : col + 128],
                                    v_sb[:, i * VW + h * 130 : i * VW + h * 130 + 129],
                                    start=(i == 0),
                                    stop=(i == g),
                                )
                        for tau in range(4):
                            g = 4 * jj + tau
                            r = rpool.tile([128, 1], f32, tag="r")
                            nc.vector.reciprocal(r[:, :], y_ps[tau][:, 128:129])
                            y_sb = ypool.tile([128, 128], bf16, tag="y")
                            nc.scalar.mul(y_sb[:, :], y_ps[tau][:, 0:128], r[:, 0:1])
                            yt_ps = paux.tile([128, 128], bf16, tag="aux")
                            nc.tensor.transpose(yt_ps[:, :], y_sb[:, :], ident[:, :])
                            nc.vector.tensor_copy(
                                yT[h][:, g * 128 : (g + 1) * 128], yt_ps[:, :]
                            )
                            if interleave_oproj:
                                oproj_tile(g)

                # ---- per-batch schedule ----
                proj_block(0)
                proj_block(1)
                proj_block(2)
                attn(0, 0, False)
                attn(0, 1, False)
                proj_block(3)
                for g in range(8):
                    oproj_tile(g)
                attn(1, 0, False)
                attn(1, 1, True)
    nc.finalize()
    return nc


def _prep_inputs(x, w_qkv, w_o, rope_cos, rope_sin):
    import ml_dtypes

    bf = ml_dtypes.bfloat16
    xTh = np.ascontiguousarray(x.reshape(N, C).T).astype(bf)
    cosT = np.ascontiguousarray(rope_cos.T)  # [64, T]
    sinT = np.ascontiguousarray(rope_sin.T)
    cos2 = np.concatenate([cosT, cosT], 0).astype(bf)  # [128, T]
    sin2 = np.concatenate([sinT, sinT], 0).astype(bf)

    r = np.arange(128)[:, None]
    c = np.arange(512)[None, :]
    singles = [((c - r) >= 128 * p).astype(np.float32) for p in range(4)]
    mk = np.concatenate(singles, axis=1).astype(bf)

    ev = np.arange(0, D, 2)
    od = np.arange(1, D, 2)
    in_maps = []
    for m in range(NCORES):
        h0, h1 = 2 * m, 2 * m + 1
        # blocks QE|QO|KE|KO; within each, cols = [head0 dims | head1 dims]
        QE = np.concatenate([w_qkv[h0 * D + ev, :], w_qkv[h1 * D + ev, :]], 0).T
        QO = np.concatenate([w_qkv[h0 * D + od, :], w_qkv[h1 * D + od, :]], 0).T
        KE = np.concatenate([w_qkv[C + h0 * D + ev, :], w_qkv[C + h1 * D + ev, :]], 0).T
        KO = np.concatenate([w_qkv[C + h0 * D + od, :], w_qkv[C + h1 * D + od, :]], 0).T
        wqk_m = np.ascontiguousarray(np.concatenate([QE, QO, KE, KO], 1)).astype(bf)
        wv_m = np.ascontiguousarray(
            w_qkv[2 * C + 2 * m * D : 2 * C + (2 * m + 2) * D, :].T
        ).astype(bf)
        wo_m = np.ascontiguousarray(w_o[:, 2 * m * D : (2 * m + 2) * D].T).astype(bf)
        in_maps.append(
            {
                "xT": xTh,
                "w_qk": wqk_m,
                "w_v": wv_m,
                "w_o": wo_m,
                "cos2": cos2,
                "sin2": sin2,
                "masks": np.ascontiguousarray(mk),
            }
        )
    return in_maps


def kernel(x, w_qkv, w_o, rope_cos, rope_sin, _trace=False):
    global _COMPILED
    x = np.asarray(x, dtype=np.float32)
    w_qkv = np.asarray(w_qkv, dtype=np.float32)
    w_o = np.asarray(w_o, dtype=np.float32)
    rope_cos = np.asarray(rope_cos, dtype=np.float32)
    rope_sin = np.asarray(rope_sin, dtype=np.float32)

    from concourse.bass_utils import run_bass_kernel_spmd

    if _COMPILED is None:
        _COMPILED = _build()
    nc = _COMPILED
    in_maps = _prep_inputs(x, w_qkv, w_o, rope_cos, rope_sin)
    res = run_bass_kernel_spmd(
        nc, in_maps, core_ids=list(range(NCORES)), trace=_trace
    )
    out = np.zeros((N, C), dtype=np.float32)
    for m in range(NCORES):
        out += res.results[m]["out_p"].astype(np.float32)
    kernel._last_results = res
    return out.reshape(B, T, C)


# revision 9
# speedup vs baseline: 1.1623x; 1.0162x over previous
"""Causal self-attention with RoPE on 8 trn2 NeuronCores.

Sharding: tensor-parallel over heads (Megatron style). 16 heads, 8 cores
-> 2 heads per core. Each core computes q/k/v for its 2 heads, causal
attention, and a partial output projection against its w_o column slice.
Host sums the 8 partial outputs (the Megatron all-reduce, done at gather).

v5 = v4 + DMA bandwidth/queue fixes:
 - All weights and x are pre-rearranged on the host into the exact SBUF
   image ([128, cols]); every load is a contiguous per-partition copy
   instead of a strided gather (the strided wqk load alone took ~29us).
 - q/k repacks batched to [64,1024] per two j-blocks and split across
   the sync (q) and scalar (k) DMA queues so they never trickle in
   behind attention's LDWEIGHTS.
 - Output DMAs alternate between the gpsimd and vector queues to halve
   the end-of-kernel drain.
 - Schedule per batch: P0 P1 R01 P2 A0 P3 R23 O(0..7) A1h0 A1h1+O(8..15).
"""

import math

import numpy as np

B, T, C, H = 2, 2048, 2048, 16
D = C // H  # 128
NCORES = 8
HPC = H // NCORES  # heads per core = 2
N = B * T  # 4096 token rows
TT = T // 128  # 16 t-tiles per batch
NB = T // 512  # 4 n/tq blocks of 512 per batch
CT = C // 128  # 16 contraction tiles

_COMPILED = None


def _build():
    import concourse.bacc as bacc
    import concourse.mybir as mybir
    import concourse.tile as tile
    from concourse.masks import make_identity

    f32 = mybir.dt.float32
    bf16 = mybir.dt.bfloat16

    nc = bacc.Bacc("TRN2", target_bir_lowering=False, debug=False)
    # all inputs pre-laid-out on host as [128, cols] SBUF images
    xTr = nc.declare_dram_parameter("xTr", [128, B * NB * CT * 512], bf16, isOutput=False)
    w_qk = nc.declare_dram_parameter("w_qk", [128, CT * 512], bf16, isOutput=False)
    w_v = nc.declare_dram_parameter("w_v", [128, CT * 256], bf16, isOutput=False)
    w_o = nc.declare_dram_parameter("w_o", [128, HPC * C], bf16, isOutput=False)
    cos2 = nc.declare_dram_parameter("cos2", [D, T], bf16, isOutput=False)
    sin2 = nc.declare_dram_parameter("sin2", [D, T], bf16, isOutput=False)
    # masks: 4 diagonal variants [128,512]; variant p keeps (c - r) >= 128*p
    masks = nc.declare_dram_parameter("masks", [128, 4 * 512], bf16, isOutput=False)
    out_p = nc.declare_dram_parameter("out_p", [N, C], bf16, isOutput=True)

    SCALE = 1.0 / math.sqrt(D)
    VW = HPC * D + 2 * HPC  # 260: per t-tile v storage [v_h0|1|pad|v_h1|1|pad]

    with tile.TileContext(nc) as tc:
        with (
            tc.tile_pool(name="wpool", bufs=1) as wpool,
            tc.tile_pool(name="xpool", bufs=2) as xpool,
            tc.tile_pool(name="eo", bufs=4) as eopool,
            tc.tile_pool(name="ropetmp", bufs=6) as tmppool,
            tc.tile_pool(name="vsb", bufs=2) as vpool,
            tc.tile_pool(name="expp", bufs=18) as exppool,
            tc.tile_pool(name="ysb", bufs=4) as ypool,
            tc.tile_pool(name="rsb", bufs=8) as rpool,
            tc.tile_pool(name="yts", bufs=3) as ytpool,
            tc.tile_pool(name="pbig", bufs=2, space="PSUM") as pbig,
            tc.tile_pool(name="paux", bufs=4, space="PSUM") as paux,
        ):
            # ---- resident weights / constants (contiguous loads) ----
            wqk_sb = wpool.tile([128, CT * 512], bf16, tag="wqk")
            for ch in range(4):
                cs = ch * 2048
                nc.sync.dma_start(
                    out=wqk_sb[:, cs : cs + 2048], in_=w_qk[:, cs : cs + 2048]
                )
            wv_sb = wpool.tile([128, CT * 256], bf16, tag="wv")
            nc.sync.dma_start(out=wv_sb[:, :], in_=w_v[:, :])
            cos_sb = wpool.tile([128, T], bf16, tag="cos")
            nc.sync.dma_start(out=cos_sb[:, :], in_=cos2[:, :])
            sin_sb = wpool.tile([128, T], bf16, tag="sin")
            nc.sync.dma_start(out=sin_sb[:, :], in_=sin2[:, :])
            mask_sb = wpool.tile([128, 4 * 512], bf16, tag="mask")
            nc.sync.dma_start(out=mask_sb[:, :], in_=masks[:, :])
            wo_sb = wpool.tile([128, HPC * C], bf16, tag="wo")
            nc.sync.dma_start(out=wo_sb[:, :], in_=w_o[:, :])
            ident = wpool.tile([128, 128], bf16, tag="ident")
            make_identity(nc, ident[:, :])

            for b in range(B):
                n0 = b * T

                v_sb = vpool.tile([128, TT * VW], bf16, tag="vsb")
                for tt in range(TT):
                    for h in range(HPC):
                        col = tt * VW + h * 130 + 128
                        nc.vector.memset(v_sb[:, col : col + 1], 1.0)

                qe2 = eopool.tile([128, T], bf16, tag="eo", name="qe2")
                qo2 = eopool.tile([128, T], bf16, tag="eo", name="qo2")
                ke2 = eopool.tile([128, T], bf16, tag="eo", name="ke2")
                ko2 = eopool.tile([128, T], bf16, tag="eo", name="ko2")
                rot = [(qe2, qo2), (ke2, ko2)]
                qhat = [eopool.tile([128, T], bf16, tag="qh", name=f"qhat{_h}") for _h in range(HPC)]
                khat = [eopool.tile([128, T], bf16, tag="qh", name=f"khat{_h}") for _h in range(HPC)]
                yT = [eopool.tile([128, T], bf16, tag="yt", name=f"yT{_h}") for _h in range(HPC)]

                def proj_block(j):
                    js = slice(j * 512, (j + 1) * 512)
                    x0 = (b * NB + j) * CT * 512
                    xt = xpool.tile([128, CT * 512], bf16, tag="xt")
                    nchunk = 4 if (b == 0 and j == 0) else 2
                    cw = CT * 512 // nchunk
                    for ch in range(nchunk):
                        nc.gpsimd.dma_start(
                            out=xt[:, ch * cw : (ch + 1) * cw],
                            in_=xTr[:, x0 + ch * cw : x0 + (ch + 1) * cw],
                        )
                    ce = cos_sb[:, js]
                    se = sin_sb[:, js]
                    for qk in range(2):  # 0 = q, 1 = k
                        ps = pbig.tile([128, 1024], f32, tag="big", name="ps_qk")
                        for c in range(CT):
                            xc = xt[:, c * 512 : (c + 1) * 512]
                            for part in range(2):
                                wsl = wqk_sb[
                                    :,
                                    c * 512
                                    + (2 * qk + part) * 128 : c * 512
                                    + (2 * qk + part + 1) * 128,
                                ]
                                nc.tensor.matmul(
                                    ps[:, part * 512 : (part + 1) * 512],
                                    wsl,
                                    xc,
                                    start=(c == 0),
                                    stop=(c == CT - 1),
                                )
                        pc = tmppool.tile([128, 1024], bf16, tag="rt", name=f"pc{qk}")
                        nc.scalar.copy(pc[:, :], ps[:, :])
                        E_sb, O_sb = pc[:, 0:512], pc[:, 512:1024]
                        dst_e, dst_o = rot[qk]
                        t1 = tmppool.tile([128, 512], bf16, tag="rt2")
                        t2 = tmppool.tile([128, 512], bf16, tag="rt2")
                        nc.vector.tensor_mul(t1[:, :], E_sb, ce)
                        nc.vector.tensor_mul(t2[:, :], O_sb, se)
                        nc.vector.tensor_sub(dst_e[:, js], t1[:, :], t2[:, :])
                        t3 = tmppool.tile([128, 512], bf16, tag="rt2")
                        t4 = tmppool.tile([128, 512], bf16, tag="rt2")
                        nc.vector.tensor_mul(t3[:, :], E_sb, se)
                        nc.vector.tensor_mul(t4[:, :], O_sb, ce)
                        nc.vector.tensor_add(dst_o[:, js], t3[:, :], t4[:, :])
                    ps_v = [
                        paux.tile([128, 256], f32, tag="aux", name=f"ps_v{_p}")
                        for _p in range(4)
                    ]
                    for c in range(CT):
                        for tl in range(4):
                            nc.tensor.matmul(
                                ps_v[tl][:, :],
                                xt[:, c * 512 + tl * 128 : c * 512 + (tl + 1) * 128],
                                wv_sb[:, c * 256 : (c + 1) * 256],
                                start=(c == 0),
                                stop=(c == CT - 1),
                            )
                    for tl in range(4):
                        tt = j * 4 + tl
                        base = tt * VW
                        for h in range(HPC):
                            nc.vector.tensor_copy(
                                v_sb[:, base + h * 130 : base + h * 130 + 128],
                                ps_v[tl][:, h * 128 : (h + 1) * 128],
                            )

                def repack2(jp):
                    # repack two j-blocks worth of rope output into per-head
                    # [d, t] layout; q on sync queue, k on scalar queue.
                    j0, j1 = jp * 1024, (jp + 1) * 1024
                    for qk in range(2):
                        dst_e, dst_o = rot[qk]
                        hat = qhat if qk == 0 else khat
                        q_eng = nc.sync if qk == 0 else nc.scalar
                        for h in range(HPC):
                            hb = 64 * h
                            q_eng.dma_start(
                                out=hat[h][0:64, j0:j1], in_=dst_e[hb : hb + 64, j0:j1]
                            )
                            q_eng.dma_start(
                                out=hat[h][64:128, j0:j1], in_=dst_o[hb : hb + 64, j0:j1]
                            )

                def oproj_tile(g):
                    tsl = slice(g * 128, (g + 1) * 128)
                    stage = ytpool.tile([128, C], bf16, tag="yo")
                    for obp in range(2):  # pairs of 512-wide o blocks
                        o_ps = pbig.tile([128, 1024], f32, tag="big", name="o_ps")
                        for ob in (2 * obp, 2 * obp + 1):
                            off = (ob - 2 * obp) * 512
                            for h in range(HPC):
                                nc.tensor.matmul(
                                    o_ps[:, off : off + 512],
                                    yT[h][:, tsl],
                                    wo_sb[:, h * C + ob * 512 : h * C + (ob + 1) * 512],
                                    start=(h == 0),
                                    stop=(h == HPC - 1),
                                )
                        nc.vector.tensor_copy(
                            stage[:, obp * 1024 : (obp + 1) * 1024], o_ps[:, :]
                        )
                    o_eng = nc.gpsimd if g % 2 == 0 else nc.sync
                    o_eng.dma_start(
                        out=out_p[n0 + g * 128 : n0 + (g + 1) * 128, :],
                        in_=stage[:, :],
                    )

                def attn(jp, h, interleave_oproj):
                    jlo, jhi = 2 * jp, 2 * jp + 1
                    exp_of = {}  # i -> (tile, combined flag)
                    for i in range(4 * jhi + 4):
                        isl = slice(i * 128, (i + 1) * 128)
                        combined = i <= 4 * jlo + 3
                        sc = pbig.tile([128, 1024], f32, tag="big", name="sc")
                        ex = exppool.tile([128, 1024], bf16, tag="ex")
                        if combined:
                            nc.tensor.matmul(
                                sc[:, 0:512], khat[h][:, isl],
                                qhat[h][:, jlo * 512 : (jlo + 1) * 512],
                                start=True, stop=True,
                            )
                            nc.tensor.matmul(
                                sc[:, 512:1024], khat[h][:, isl],
                                qhat[h][:, jhi * 512 : (jhi + 1) * 512],
                                start=True, stop=True,
                            )
                            nc.scalar.activation(
                                ex[:, :], sc[:, :],
                                mybir.ActivationFunctionType.Exp, scale=SCALE,
                            )
                            p = i - 4 * jlo
                            if p >= 0:
                                nc.vector.tensor_mul(
                                    ex[:, 0:512], ex[:, 0:512],
                                    mask_sb[:, p * 512 : (p + 1) * 512],
                                )
                            exp_of[i] = (ex, True)
                        else:
                            nc.tensor.matmul(
                                sc[:, 0:512], khat[h][:, isl],
                                qhat[h][:, jhi * 512 : (jhi + 1) * 512],
                                start=True, stop=True,
                            )
                            nc.scalar.activation(
                                ex[:, 0:512], sc[:, 0:512],
                                mybir.ActivationFunctionType.Exp, scale=SCALE,
                            )
                            p = i - 4 * jhi
                            if p >= 0:
                                nc.vector.tensor_mul(
                                    ex[:, 0:512], ex[:, 0:512],
                                    mask_sb[:, p * 512 : (p + 1) * 512],
                                )
                            exp_of[i] = (ex, False)

                    for jj in (jlo, jhi):
                        half = 0 if jj == jlo else 512
                        y_ps = [
                            paux.tile([128, 129], f32, tag="aux", name=f"y_ps{_p}")
                            for _p in range(4)
                        ]
                        for tau in range(4):
                            g = 4 * jj + tau
                            for i in range(g + 1):
                                ex, comb = exp_of[i]
                                col = (half if comb else 0) + tau * 128
                                nc.tensor.matmul(
                                    y_ps[tau][:, :],
                                    ex[:, col : col + 128],
                                    v_sb[:, i * VW + h * 130 : i * VW + h * 130 + 129],
                                    start=(i == 0),
                                    stop=(i == g),
                                )
                        for tau in range(4):
                            g = 4 * jj + tau
                            r = rpool.tile([128, 1], f32, tag="r")
                            nc.vector.reciprocal(r[:, :], y_ps[tau][:, 128:129])
                            y_sb = ypool.tile([128, 128], bf16, tag="y")
                            nc.scalar.mul(y_sb[:, :], y_ps[tau][:, 0:128], r[:, 0:1])
                            yt_ps = paux.tile([128, 128], bf16, tag="aux")
                            nc.tensor.transpose(yt_ps[:, :], y_sb[:, :], ident[:, :])
                            nc.vector.tensor_copy(
                                yT[h][:, g * 128 : (g + 1) * 128], yt_ps[:, :]
                            )
                            if interleave_oproj:
                                oproj_tile(g)

                # ---- per-batch schedule ----
                proj_block(0)
                proj_block(1)
                repack2(0)
                proj_block(2)
                attn(0, 0, False)
                attn(0, 1, False)
                proj_block(3)
                repack2(1)
                for g in range(8):
                    oproj_tile(g)
                attn(1, 0, False)
                attn(1, 1, True)
    nc.finalize()
    return nc


def _prep_inputs(x, w_qkv, w_o, rope_cos, rope_sin):
    import ml_dtypes

    bf = ml_dtypes.bfloat16
    # x pre-tiled: xTr[p, ((b*NB + j)*CT + c)*512 + n] = x[b, j*512+n, c*128+p]
    xb = x.reshape(B, NB, 512, CT, 128)  # [b, j, n, c, p]
    xTr = np.ascontiguousarray(xb.transpose(4, 0, 1, 3, 2).reshape(128, -1)).astype(bf)
    cosT = np.ascontiguousarray(rope_cos.T)  # [64, T]
    sinT = np.ascontiguousarray(rope_sin.T)
    cos2 = np.concatenate([cosT, cosT], 0).astype(bf)  # [128, T]
    sin2 = np.concatenate([sinT, sinT], 0).astype(bf)

    r = np.arange(128)[:, None]
    c = np.arange(512)[None, :]
    singles = [((c - r) >= 128 * p).astype(np.float32) for p in range(4)]
    mk = np.concatenate(singles, axis=1).astype(bf)

    def sbuf_image_kt(w, kt, width):
        # w: [kt*128, width] -> [128, kt*width] with col kt*width… image
        # img[p, k*width + e] = w[k*128 + p, e]
        return np.ascontiguousarray(
            w.reshape(kt, 128, width).transpose(1, 0, 2).reshape(128, kt * width)
        )

    ev = np.arange(0, D, 2)
    od = np.arange(1, D, 2)
    in_maps = []
    for m in range(NCORES):
        h0, h1 = 2 * m, 2 * m + 1
        # blocks QE|QO|KE|KO; within each, cols = [head0 dims | head1 dims]
        QE = np.concatenate([w_qkv[h0 * D + ev, :], w_qkv[h1 * D + ev, :]], 0).T
        QO = np.concatenate([w_qkv[h0 * D + od, :], w_qkv[h1 * D + od, :]], 0).T
        KE = np.concatenate([w_qkv[C + h0 * D + ev, :], w_qkv[C + h1 * D + ev, :]], 0).T
        KO = np.concatenate([w_qkv[C + h0 * D + od, :], w_qkv[C + h1 * D + od, :]], 0).T
        wqk_m = sbuf_image_kt(np.concatenate([QE, QO, KE, KO], 1), CT, 512).astype(bf)
        wv_m = sbuf_image_kt(
            w_qkv[2 * C + 2 * m * D : 2 * C + (2 * m + 2) * D, :].T, CT, 256
        ).astype(bf)
        wo_m = sbuf_image_kt(
            w_o[:, 2 * m * D : (2 * m + 2) * D].T, HPC, C
        ).astype(bf)
        in_maps.append(
            {
                "xTr": xTr,
                "w_qk": wqk_m,
                "w_v": wv_m,
                "w_o": wo_m,
                "cos2": cos2,
                "sin2": sin2,
                "masks": np.ascontiguousarray(mk),
            }
        )
    return in_maps


def kernel(x, w_qkv, w_o, rope_cos, rope_sin, _trace=False):
    global _COMPILED
    x = np.asarray(x, dtype=np.float32)
    w_qkv = np.asarray(w_qkv, dtype=np.float32)
    w_o = np.asarray(w_o, dtype=np.float32)
    rope_cos = np.asarray(rope_cos, dtype=np.float32)
    rope_sin = np.asarray(rope_sin, dtype=np.float32)

    from concourse.bass_utils import run_bass_kernel_spmd

    if _COMPILED is None:
        _COMPILED = _build()
    nc = _COMPILED
    in_maps = _prep_inputs(x, w_qkv, w_o, rope_cos, rope_sin)
    res = run_bass_kernel_spmd(
        nc, in_maps, core_ids=list(range(NCORES)), trace=_trace
    )
    out = np.zeros((N, C), dtype=np.float32)
    for m in range(NCORES):
        out += res.results[m]["out_p"].astype(np.float32)
    kernel._last_results = res
    return out.reshape(B, T, C)


# revision 15
# speedup vs baseline: 1.1748x; 1.0107x over previous
"""Causal self-attention with RoPE on 8 trn2 NeuronCores.

Sharding: tensor-parallel over heads (Megatron style). 16 heads, 8 cores
-> 2 heads per core. Each core computes q/k/v for its 2 heads, causal
attention, and a partial output projection against its w_o column slice.
Host sums the 8 partial outputs (the Megatron all-reduce, done at gather).

v5 = v4 + DMA bandwidth/queue fixes:
 - All weights and x are pre-rearranged on the host into the exact SBUF
   image ([128, cols]); every load is a contiguous per-partition copy
   instead of a strided gather (the strided wqk load alone took ~29us).
 - q/k repacks batched to [64,1024] per two j-blocks and split across
   the sync (q) and scalar (k) DMA queues so they never trickle in
   behind attention's LDWEIGHTS.
 - Output DMAs alternate between the gpsimd and vector queues to halve
   the end-of-kernel drain.
 - Schedule per batch: P0 P1 R01 P2 A0 P3 R23 O(0..7) A1h0 A1h1+O(8..15).
"""

import math

import numpy as np

B, T, C, H = 2, 2048, 2048, 16
D = C // H  # 128
NCORES = 8
HPC = H // NCORES  # heads per core = 2
N = B * T  # 4096 token rows
TT = T // 128  # 16 t-tiles per batch
NB = T // 512  # 4 n/tq blocks of 512 per batch
CT = C // 128  # 16 contraction tiles

_COMPILED = None


def _build():
    import concourse.bacc as bacc
    import concourse.mybir as mybir
    import concourse.tile as tile
    from concourse.masks import make_identity

    f32 = mybir.dt.float32
    bf16 = mybir.dt.bfloat16

    nc = bacc.Bacc("TRN2", target_bir_lowering=False, debug=False)
    # all inputs pre-laid-out on host as [128, cols] SBUF images
    xTr = nc.declare_dram_parameter("xTr", [128, B * NB * CT * 512], bf16, isOutput=False)
    w_qk = nc.declare_dram_parameter("w_qk", [128, CT * 512], bf16, isOutput=False)
    w_v = nc.declare_dram_parameter("w_v", [128, CT * 256], bf16, isOutput=False)
    w_o = nc.declare_dram_parameter("w_o", [128, HPC * C], bf16, isOutput=False)
    cos2 = nc.declare_dram_parameter("cos2", [D, T], bf16, isOutput=False)
    sin2 = nc.declare_dram_parameter("sin2", [D, T], bf16, isOutput=False)
    # masks: 4 diagonal variants [128,512]; variant p keeps (c - r) >= 128*p
    masks = nc.declare_dram_parameter("masks", [128, 4 * 512], bf16, isOutput=False)
    out_p = nc.declare_dram_parameter("out_p", [N, C], bf16, isOutput=True)

    SCALE = 1.0 / math.sqrt(D)
    VW = HPC * D + 2 * HPC  # 260: per t-tile v storage [v_h0|1|pad|v_h1|1|pad]

    with tile.TileContext(nc) as tc:
        with (
            tc.tile_pool(name="wpool", bufs=1) as wpool,
            tc.tile_pool(name="xpool", bufs=2) as xpool,
            tc.tile_pool(name="eo", bufs=4) as eopool,
            tc.tile_pool(name="ropetmp", bufs=6) as tmppool,
            tc.tile_pool(name="vsb", bufs=2) as vpool,
            tc.tile_pool(name="expp", bufs=18) as exppool,
            tc.tile_pool(name="ysb", bufs=4) as ypool,
            tc.tile_pool(name="rsb", bufs=8) as rpool,
            tc.tile_pool(name="yts", bufs=3) as ytpool,
            tc.tile_pool(name="pbig", bufs=2, space="PSUM") as pbig,
            tc.tile_pool(name="paux", bufs=4, space="PSUM") as paux,
        ):
            # ---- resident weights / constants (contiguous loads) ----
            wqk_sb = wpool.tile([128, CT * 512], bf16, tag="wqk")
            wqk_cuts = [0, 512, 1024, 2048, 4096, 6144, 8192]
            for cs, ce_ in zip(wqk_cuts, wqk_cuts[1:]):
                nc.sync.dma_start(out=wqk_sb[:, cs:ce_], in_=w_qk[:, cs:ce_])
            wv_sb = wpool.tile([128, CT * 256], bf16, tag="wv")
            nc.sync.dma_start(out=wv_sb[:, :], in_=w_v[:, :])
            cos_sb = wpool.tile([128, T], bf16, tag="cos")
            nc.sync.dma_start(out=cos_sb[:, :], in_=cos2[:, :])
            sin_sb = wpool.tile([128, T], bf16, tag="sin")
            nc.sync.dma_start(out=sin_sb[:, :], in_=sin2[:, :])
            mask_sb = wpool.tile([128, 4 * 512], bf16, tag="mask")
            nc.sync.dma_start(out=mask_sb[:, :], in_=masks[:, :])
            wo_sb = wpool.tile([128, HPC * C], bf16, tag="wo")
            nc.sync.dma_start(out=wo_sb[:, :], in_=w_o[:, :])
            ident = wpool.tile([128, 128], bf16, tag="ident")
            make_identity(nc, ident[:, :])

            for b in range(B):
                n0 = b * T

                v_sb = vpool.tile([128, TT * VW], bf16, tag="vsb")
                for tt in range(TT):
                    for h in range(HPC):
                        col = tt * VW + h * 130 + 128
                        nc.vector.memset(v_sb[:, col : col + 1], 1.0)

                qe2 = eopool.tile([128, T], bf16, tag="eo", name="qe2")
                qo2 = eopool.tile([128, T], bf16, tag="eo", name="qo2")
                ke2 = eopool.tile([128, T], bf16, tag="eo", name="ke2")
                ko2 = eopool.tile([128, T], bf16, tag="eo", name="ko2")
                rot = [(qe2, qo2), (ke2, ko2)]
                qhat = [eopool.tile([128, T], bf16, tag="qh", name=f"qhat{_h}") for _h in range(HPC)]
                khat = [eopool.tile([128, T], bf16, tag="qh", name=f"khat{_h}") for _h in range(HPC)]
                yT = [eopool.tile([128, T], bf16, tag="yt", name=f"yT{_h}") for _h in range(HPC)]

                def proj_block(j):
                    js = slice(j * 512, (j + 1) * 512)
                    x0 = (b * NB + j) * CT * 512
                    xt = xpool.tile([128, CT * 512], bf16, tag="xt")
                    if b == 0 and j == 0:
                        cuts = [0, 1024, 2048, 4096, 8192]
                    else:
                        cuts = [0, 4096, 8192]
                    for cs, ce_ in zip(cuts, cuts[1:]):
                        nc.gpsimd.dma_start(
                            out=xt[:, cs:ce_], in_=xTr[:, x0 + cs : x0 + ce_]
                        )
                    ce = cos_sb[:, js]
                    se = sin_sb[:, js]
                    for qk in range(2):  # 0 = q, 1 = k
                        ps = pbig.tile([128, 1024], f32, tag="big", name="ps_qk")
                        for c in range(CT):
                            xc = xt[:, c * 512 : (c + 1) * 512]
                            for part in range(2):
                                wsl = wqk_sb[
                                    :,
                                    c * 512
                                    + (2 * qk + part) * 128 : c * 512
                                    + (2 * qk + part + 1) * 128,
                                ]
                                nc.tensor.matmul(
                                    ps[:, part * 512 : (part + 1) * 512],
                                    wsl,
                                    xc,
                                    start=(c == 0),
                                    stop=(c == CT - 1),
                                )
                        pc = tmppool.tile([128, 1024], bf16, tag="rt", name=f"pc{qk}")
                        nc.scalar.copy(pc[:, :], ps[:, :])
                        E_sb, O_sb = pc[:, 0:512], pc[:, 512:1024]
                        dst_e, dst_o = rot[qk]
                        t1 = tmppool.tile([128, 512], bf16, tag="rt2")
                        t2 = tmppool.tile([128, 512], bf16, tag="rt2")
                        nc.vector.tensor_mul(t1[:, :], E_sb, ce)
                        nc.vector.tensor_mul(t2[:, :], O_sb, se)
                        nc.vector.tensor_sub(dst_e[:, js], t1[:, :], t2[:, :])
                        t3 = tmppool.tile([128, 512], bf16, tag="rt2")
                        t4 = tmppool.tile([128, 512], bf16, tag="rt2")
                        nc.vector.tensor_mul(t3[:, :], E_sb, se)
                        nc.vector.tensor_mul(t4[:, :], O_sb, ce)
                        nc.vector.tensor_add(dst_o[:, js], t3[:, :], t4[:, :])
                        # repack this j-block right away; q on sync queue,
                        # k on the scalar queue so neither trickles in late
                        hat = qhat if qk == 0 else khat
                        q_eng = nc.sync if qk == 0 else nc.scalar
                        for h in range(HPC):
                            hb = 64 * h
                            q_eng.dma_start(
                                out=hat[h][0:64, js], in_=dst_e[hb : hb + 64, js]
                            )
                            q_eng.dma_start(
                                out=hat[h][64:128, js], in_=dst_o[hb : hb + 64, js]
                            )
                    ps_v = [
                        paux.tile([128, 256], f32, tag="aux", name=f"ps_v{_p}")
                        for _p in range(4)
                    ]
                    for c in range(CT):
                        for tl in range(4):
                            nc.tensor.matmul(
                                ps_v[tl][:, :],
                                xt[:, c * 512 + tl * 128 : c * 512 + (tl + 1) * 128],
                                wv_sb[:, c * 256 : (c + 1) * 256],
                                start=(c == 0),
                                stop=(c == CT - 1),
                            )
                    for tl in range(4):
                        tt = j * 4 + tl
                        base = tt * VW
                        for h in range(HPC):
                            nc.vector.tensor_copy(
                                v_sb[:, base + h * 130 : base + h * 130 + 128],
                                ps_v[tl][:, h * 128 : (h + 1) * 128],
                            )

                def oproj_tile(g):
                    tsl = slice(g * 128, (g + 1) * 128)
                    stage = ytpool.tile([128, C], bf16, tag="yo")
                    for obp in range(2):  # pairs of 512-wide o blocks
                        o_ps = pbig.tile([128, 1024], f32, tag="big", name="o_ps")
                        for ob in (2 * obp, 2 * obp + 1):
                            off = (ob - 2 * obp) * 512
                            for h in range(HPC):
                                nc.tensor.matmul(
                                    o_ps[:, off : off + 512],
                                    yT[h][:, tsl],
                                    wo_sb[:, h * C + ob * 512 : h * C + (ob + 1) * 512],
                                    start=(h == 0),
                                    stop=(h == HPC - 1),
                                )
                        nc.vector.tensor_copy(
                            stage[:, obp * 1024 : (obp + 1) * 1024], o_ps[:, :]
                        )
                    rows = slice(n0 + g * 128, n0 + (g + 1) * 128)
                    if b == B - 1 and g == TT - 1:
                        # split the very last write across two queues to
                        # shorten the end-of-kernel drain
                        nc.gpsimd.dma_start(out=out_p[rows, 0:1024], in_=stage[:, 0:1024])
                        nc.sync.dma_start(out=out_p[rows, 1024:2048], in_=stage[:, 1024:2048])
                    else:
                        o_eng = nc.gpsimd if g % 2 == 0 else nc.sync
                        o_eng.dma_start(out=out_p[rows, :], in_=stage[:, :])

                def attn(jp, h, interleave_oproj):
                    jlo, jhi = 2 * jp, 2 * jp + 1
                    exp_of = {}  # i -> (tile, combined flag)
                    for i in range(4 * jhi + 4):
                        isl = slice(i * 128, (i + 1) * 128)
                        combined = i <= 4 * jlo + 3
                        sc = pbig.tile([128, 1024], f32, tag="big", name="sc")
                        ex = exppool.tile([128, 1024], bf16, tag="ex")
                        if combined:
                            nc.tensor.matmul(
                                sc[:, 0:512], khat[h][:, isl],
                                qhat[h][:, jlo * 512 : (jlo + 1) * 512],
                                start=True, stop=True,
                            )
                            nc.tensor.matmul(
                                sc[:, 512:1024], khat[h][:, isl],
                                qhat[h][:, jhi * 512 : (jhi + 1) * 512],
                                start=True, stop=True,
                            )
                            nc.scalar.activation(
                                ex[:, :], sc[:, :],
                                mybir.ActivationFunctionType.Exp, scale=SCALE,
                            )
                            p = i - 4 * jlo
                            if p >= 0:
                                nc.vector.tensor_mul(
                                    ex[:, 0:512], ex[:, 0:512],
                                    mask_sb[:, p * 512 : (p + 1) * 512],
                                )
                            exp_of[i] = (ex, True)
                        else:
                            nc.tensor.matmul(
                                sc[:, 0:512], khat[h][:, isl],
                                qhat[h][:, jhi * 512 : (jhi + 1) * 512],
                                start=True, stop=True,
                            )
                            nc.scalar.activation(
                                ex[:, 0:512], sc[:, 0:512],
                                mybir.ActivationFunctionType.Exp, scale=SCALE,
                            )
                            p = i - 4 * jhi
                            if p >= 0:
                                nc.vector.tensor_mul(
                                    ex[:, 0:512], ex[:, 0:512],
                                    mask_sb[:, p * 512 : (p + 1) * 512],
                                )
                            exp_of[i] = (ex, False)

                    for jj in (jlo, jhi):
                        half = 0 if jj == jlo else 512
                        y_ps = [
                            paux.tile([128, 129], f32, tag="aux", name=f"y_ps{_p}")
                            for _p in range(4)
                        ]
                        for tau in range(4):
                            g = 4 * jj + tau
                            for i in range(g + 1):
                                ex, comb = exp_of[i]
                                col = (half if comb else 0) + tau * 128
                                nc.tensor.matmul(
                                    y_ps[tau][:, :],
                                    ex[:, col : col + 128],
                                    v_sb[:, i * VW + h * 130 : i * VW + h * 130 + 129],
                                    start=(i == 0),
                                    stop=(i == g),
                                )
                        for tau in range(4):
                            g = 4 * jj + tau
                            r = rpool.tile([128, 1], f32, tag="r")
                            nc.vector.reciprocal(r[:, :], y_ps[tau][:, 128:129])
                            y_sb = ypool.tile([128, 128], bf16, tag="y")
                            nc.scalar.mul(y_sb[:, :], y_ps[tau][:, 0:128], r[:, 0:1])
                            yt_ps = paux.tile([128, 128], bf16, tag="aux")
                            nc.tensor.transpose(yt_ps[:, :], y_sb[:, :], ident[:, :])
                            nc.vector.tensor_copy(
                                yT[h][:, g * 128 : (g + 1) * 128], yt_ps[:, :]
                            )
                            if interleave_oproj:
                                oproj_tile(g)

                # ---- per-batch schedule ----
                proj_block(0)
                proj_block(1)
                proj_block(2)
                attn(0, 0, False)
                attn(0, 1, False)
                proj_block(3)
                for g in range(8):
                    oproj_tile(g)
                attn(1, 0, False)
                attn(1, 1, True)
    nc.finalize()
    return nc


def _prep_inputs(x, w_qkv, w_o, rope_cos, rope_sin):
    import ml_dtypes

    bf = ml_dtypes.bfloat16
    # x pre-tiled: xTr[p, ((b*NB + j)*CT + c)*512 + n] = x[b, j*512+n, c*128+p]
    xb = x.reshape(B, NB, 512, CT, 128)  # [b, j, n, c, p]
    xTr = np.ascontiguousarray(xb.transpose(4, 0, 1, 3, 2).reshape(128, -1)).astype(bf)
    cosT = np.ascontiguousarray(rope_cos.T)  # [64, T]
    sinT = np.ascontiguousarray(rope_sin.T)
    cos2 = np.concatenate([cosT, cosT], 0).astype(bf)  # [128, T]
    sin2 = np.concatenate([sinT, sinT], 0).astype(bf)

    r = np.arange(128)[:, None]
    c = np.arange(512)[None, :]
    singles = [((c - r) >= 128 * p).astype(np.float32) for p in range(4)]
    mk = np.concatenate(singles, axis=1).astype(bf)

    def sbuf_image_kt(w, kt, width):
        # w: [kt*128, width] -> [128, kt*width] with col kt*width… image
        # img[p, k*width + e] = w[k*128 + p, e]
        return np.ascontiguousarray(
            w.reshape(kt, 128, width).transpose(1, 0, 2).reshape(128, kt * width)
        )

    ev = np.arange(0, D, 2)
    od = np.arange(1, D, 2)
    in_maps = []
    for m in range(NCORES):
        h0, h1 = 2 * m, 2 * m + 1
        # blocks QE|QO|KE|KO; within each, cols = [head0 dims | head1 dims]
        QE = np.concatenate([w_qkv[h0 * D + ev, :], w_qkv[h1 * D + ev, :]], 0).T
        QO = np.concatenate([w_qkv[h0 * D + od, :], w_qkv[h1 * D + od, :]], 0).T
        KE = np.concatenate([w_qkv[C + h0 * D + ev, :], w_qkv[C + h1 * D + ev, :]], 0).T
        KO = np.concatenate([w_qkv[C + h0 * D + od, :], w_qkv[C + h1 * D + od, :]], 0).T
        wqk_m = sbuf_image_kt(np.concatenate([QE, QO, KE, KO], 1), CT, 512).astype(bf)
        wv_m = sbuf_image_kt(
            w_qkv[2 * C + 2 * m * D : 2 * C + (2 * m + 2) * D, :].T, CT, 256
        ).astype(bf)
        wo_m = sbuf_image_kt(
            w_o[:, 2 * m * D : (2 * m + 2) * D].T, HPC, C
        ).astype(bf)
        in_maps.append(
            {
                "xTr": xTr,
                "w_qk": wqk_m,
                "w_v": wv_m,
                "w_o": wo_m,
                "cos2": cos2,
                "sin2": sin2,
                "masks": np.ascontiguousarray(mk),
            }
        )
    return in_maps


def kernel(x, w_qkv, w_o, rope_cos, rope_sin, _trace=False):
    global _COMPILED
    x = np.asarray(x, dtype=np.float32)
    w_qkv = np.asarray(w_qkv, dtype=np.float32)
    w_o = np.asarray(w_o, dtype=np.float32)
    rope_cos = np.asarray(rope_cos, dtype=np.float32)
    rope_sin = np.asarray(rope_sin, dtype=np.float32)

    from concourse.bass_utils import run_bass_kernel_spmd

    if _COMPILED is None:
        _COMPILED = _build()
    nc = _COMPILED
    in_maps = _prep_inputs(x, w_qkv, w_o, rope_cos, rope_sin)
    res = run_bass_kernel_spmd(
        nc, in_maps, core_ids=list(range(NCORES)), trace=_trace
    )
    out = np.zeros((N, C), dtype=np.float32)
    for m in range(NCORES):
        out += res.results[m]["out_p"].astype(np.float32)
    kernel._last_results = res
    return out.reshape(B, T, C)
